# revision 6
# baseline (speedup 1.0000x reference)
"""Multi-head self-attention (B=8, N=1024, E=768, H=12, D=64) on 8 TRN2
NeuronCores, data-parallel over the batch dimension (one batch element per
core).

Per-core pipeline (all matmuls in float32r = full-rate fp32 on the PE):
  1. Q^T / K^T projections producing head-reinterpreted transposed layouts
     directly via strided PSUM evictions (the torch code reshapes [N, E] ->
     [H, N, D] without transpose, so head slabs are contiguous runs of the
     flat projection output; feature-chunk j and token n interleave with
     c = 12 n + j).
  2. V projection in natural layout, bounced through DRAM to reload as
     m-chunk tiles (with a ones column appended for the softmax denominator).
  3. Per head: S^T = K_h Q_h^T on the PE (64-partition bands, two heads per
     128 partitions), exp on the scalar engine (scale=1/8 fused, no max
     subtraction needed: logits are O(1)), P~^T V via PE accumulation, giving
     unnormalized O^T plus denominators.
  4. Batched reciprocal of all 24 denominator rows on the vector engine,
     broadcast back via DRAM, in-place normalization of O^T.
  5. Output projection + bias, DMA out.
"""
import sys
import contextlib

sys.path.insert(0, "/opt/trn_rl_repo")

import numpy as np
import concourse.bass as bass
import concourse.tile as tile
from concourse import mybir, bacc
from concourse.bass_utils import run_bass_kernel_spmd

F32 = mybir.dt.float32
F32R = mybir.dt.float32r
AF = mybir.ActivationFunctionType

N, E, H, D = 1024, 768, 12, 64
NCH = N // 128          # 8 token chunks
ECH = E // 128          # 6 contraction chunks
MCH = N // 128          # 8 key-chunks per head
NW = (0, 512)           # query windows (start), each 512 wide
GW = ((0, 512), (512, 256))  # output-feature windows

_CACHE = {}


def _ceil_div(a, b):
    return -(-a // b)


def _head_segments(j, w):
    """Split the stride-12 evict of feature-row j, token window w into
    per-head segments. Yields (h, nlo, nhi, m0): tokens n in [nlo, nhi) of
    window w land at head h, in-head positions m0, m0+12, ..."""
    n0, n1 = 512 * w, 512 * w + 512
    c_lo, c_hi = 12 * n0 + j, 12 * (n1 - 1) + j
    for h in range(c_lo // 1024, c_hi // 1024 + 1):
        nlo = max(n0, _ceil_div(1024 * h - j, 12))
        nhi = min(n1, _ceil_div(1024 * (h + 1) - j, 12))
        if nlo < nhi:
            yield h, nlo, nhi, 12 * nlo + j - 1024 * h


def _build(ctx, tc, io):
    nc = tc.nc
    x_t, wq_t, wk_t, wv_t, wo_t, b_o, y, v_scr, den_d, denr_d, ones_d = io

    pw = ctx.enter_context(tc.tile_pool(name="pw", bufs=2))
    pbig = ctx.enter_context(tc.tile_pool(name="pbig", bufs=1))
    pqk = ctx.enter_context(tc.tile_pool(name="pqk", bufs=1))
    pes = ctx.enter_context(tc.tile_pool(name="pes", bufs=8))
    pvh = ctx.enter_context(tc.tile_pool(name="pvh", bufs=3))
    pvn = ctx.enter_context(tc.tile_pool(name="pvn", bufs=2))
    pys = ctx.enter_context(tc.tile_pool(name="pys", bufs=2))
    psml = ctx.enter_context(tc.tile_pool(name="psml", bufs=4))
    pstg = ctx.enter_context(tc.tile_pool(name="pstg", bufs=2))
    pp = ctx.enter_context(tc.tile_pool(name="pp", bufs=2, space="PSUM"))
    sp = ctx.enter_context(tc.tile_pool(name="sp", bufs=2, space="PSUM"))
    vp = ctx.enter_context(tc.tile_pool(name="vp", bufs=2, space="PSUM"))

    # ---- input loads -----------------------------------------------------
    bias_b = psml.tile([128, E], F32, tag="bias")
    nc.gpsimd.dma_start(
        out=bias_b[:],
        in_=bass.AP(tensor=b_o.tensor, offset=0, ap=[[0, 128], [1, E]]),
    )
    xs = pbig.tile([128, ECH, N], F32R, tag="big")
    for ec in range(ECH):
        nc.sync.dma_start(xs[:, ec, :], x_t[128 * ec:128 * ec + 128, :])

    def load_w(src):
        ws = pw.tile([128, ECH, E], F32R, tag="w")
        for ec in range(ECH):
            nc.sync.dma_start(ws[:, ec, :], src[128 * ec:128 * ec + 128, :])
        return ws

    wq_s = load_w(wq_t)
    wk_s = load_w(wk_t)

    # qk: [128, 12 slots, 1024]; head h lives in partition band 64*(h%2),
    # slot h//2 holds Q_h^T, slot 6 + h//2 holds K_h^T (columns = in-head m).
    qk = pqk.tile([128, 12, N], F32R, tag="qk")

    # ---- Q^T / K^T projections with head-scatter evictions ---------------
    def qk_proj(ws, slot_base):
        for u in range(ECH):
            for w in range(2):
                ps = pp.tile([128, 512], F32, tag="pp")
                for ec in range(ECH):
                    nc.tensor.matmul(
                        ps[:], ws[:, ec, 128 * u:128 * u + 128],
                        xs[:, ec, 512 * w:512 * w + 512],
                        start=(ec == 0), stop=(ec == ECH - 1),
                    )
                for jj in range(2):
                    j = 2 * u + jj
                    for h, nlo, nhi, m0 in _head_segments(j, w):
                        b, cnt = h % 2, nhi - nlo
                        dst = qk[64 * b:64 * b + 64, slot_base + h // 2,
                                 m0:m0 + 12 * (cnt - 1) + 1:12]
                        nc.vector.tensor_copy(
                            dst, ps[64 * jj:64 * jj + 64, nlo - 512 * w:nhi - 512 * w])

    qk_proj(wq_s, 0)
    qk_proj(wk_s, 6)

    # ---- V projection (natural layout) -> DRAM bounce --------------------
    wv_s = load_w(wv_t)
    for t in range(NCH):
        vn = pvn.tile([128, E], F32R, tag="vn")
        for g0, gsz in GW:
            ps = pp.tile([128, 512], F32, tag="pp")
            for ec in range(ECH):
                nc.tensor.matmul(
                    ps[:, :gsz], xs[:, ec, 128 * t:128 * t + 128],
                    wv_s[:, ec, g0:g0 + gsz],
                    start=(ec == 0), stop=(ec == ECH - 1),
                )
            nc.vector.tensor_copy(vn[:, g0:g0 + gsz], ps[:, :gsz])
        nc.sync.dma_start(v_scr[128 * t:128 * t + 128, :], vn[:])

    wo_s = load_w(wo_t)  # prefetch for the final projection

    # ---- attention per head ----------------------------------------------
    v_flat = v_scr.rearrange("n e -> (n e)")
    # oct: O_concat^T [128, 6, 1024]; feature F = 64 h + d -> partition
    # 64*(h%2) + d, slot h//2; columns = query index.
    oct = pbig.tile([128, ECH, N], F32R, tag="big")

    for h in range(H):
        b = h % 2
        base = 64 * b
        vh = pvh.tile([128, MCH, D + 1], F32R, tag="vh")
        nc.gpsimd.dma_start(
            out=vh[:, :, D:D + 1],
            in_=bass.AP(tensor=ones_d.tensor, offset=0,
                        ap=[[0, 128], [1, MCH], [0, 1]]),
        )
        nc.sync.dma_start(
            vh[:, :, 0:D],
            v_flat[65536 * h:65536 * (h + 1)].rearrange(
                "(t p d) -> p t d", t=MCH, p=128, d=D),
        )
        es_tiles = []
        for mc in range(MCH):
            st = sp.tile([128, 2 * 512], F32, tag="sp")
            for w in range(2):
                nc.tensor.matmul(
                    st[:, 512 * w:512 * w + 512],
                    qk[base:base + 64, 6 + h // 2, 128 * mc:128 * mc + 128],
                    qk[base:base + 64, h // 2, 512 * w:512 * w + 512],
                    start=True, stop=True, tile_position=(base, 0),
                )
            es = pes.tile([128, 2 * 512], F32R, tag="es")
            nc.scalar.activation(es[:], st[:], AF.Exp, scale=1.0 / np.sqrt(D))
            es_tiles.append(es)
        for w in range(2):
            pv = vp.tile([D + 1, 512], F32, tag="vp")
            for mc in range(MCH):
                nc.tensor.matmul(
                    pv[:], vh[:, mc, :], es_tiles[mc][:, 512 * w:512 * w + 512],
                    start=(mc == 0), stop=(mc == MCH - 1),
                )
            stg = pstg.tile([1, 512], F32, tag="stg")
            nc.vector.tensor_copy(stg[:], pv[D:D + 1, :])
            nc.sync.dma_start(den_d[2 * h + w, :], stg[:])
            nc.vector.tensor_copy(
                oct[base:base + 64, h // 2, 512 * w:512 * w + 512], pv[0:D, :])

    # ---- softmax normalization -------------------------------------------
    denp = psml.tile([24, 512], F32, tag="denp")
    nc.sync.dma_start(denp[:], den_d)
    denr = psml.tile([24, 512], F32, tag="denr")
    nc.vector.reciprocal(denr[:], denp[:])
    nc.sync.dma_start(denr_d, denr[:])
    for h in range(H):
        base = 64 * (h % 2)
        for w in range(2):
            dben = psml.tile([128, 512], F32, tag="dben")
            nc.gpsimd.dma_start(
                out=dben[:],
                in_=bass.AP(tensor=denr_d.tensor, offset=512 * (2 * h + w),
                            ap=[[0, 128], [1, 512]]),
            )
            slab = oct[base:base + 64, h // 2, 512 * w:512 * w + 512]
            nc.vector.tensor_mul(slab, slab, dben[base:base + 64, :])

    # ---- output projection + bias ----------------------------------------
    for t in range(NCH):
        ys = pys.tile([128, E], F32, tag="ys")
        for g0, gsz in GW:
            ps = pp.tile([128, 512], F32, tag="pp")
            for u in range(ECH):
                nc.tensor.matmul(
                    ps[:, :gsz], oct[:, u, 128 * t:128 * t + 128],
                    wo_s[:, u, g0:g0 + gsz],
                    start=(u == 0), stop=(u == ECH - 1),
                )
            nc.vector.tensor_add(ys[:, g0:g0 + gsz], ps[:, :gsz],
                                 bias_b[:, g0:g0 + gsz])
        nc.sync.dma_start(y[128 * t:128 * t + 128, :], ys[:])


def build_nc():
    if "nc" in _CACHE:
        return _CACHE["nc"]
    nc = bacc.Bacc("TRN2", target_bir_lowering=False, debug=False,
                   num_devices=8)
    io = (
        nc.dram_tensor("x_t", [E, N], F32R, kind="ExternalInput").ap(),
        nc.dram_tensor("wq_t", [E, E], F32R, kind="ExternalInput").ap(),
        nc.dram_tensor("wk_t", [E, E], F32R, kind="ExternalInput").ap(),
        nc.dram_tensor("wv_t", [E, E], F32R, kind="ExternalInput").ap(),
        nc.dram_tensor("wo_t", [E, E], F32R, kind="ExternalInput").ap(),
        nc.dram_tensor("b_o", [E], F32, kind="ExternalInput").ap(),
        nc.dram_tensor("y", [N, E], F32, kind="ExternalOutput").ap(),
        nc.dram_tensor("v_scr", [N, E], F32R).ap(),
        nc.dram_tensor("den_d", [24, 512], F32).ap(),
        nc.dram_tensor("denr_d", [24, 512], F32).ap(),
        nc.dram_tensor("ones_d", [8], F32R, kind="ExternalInput").ap(),
    )
    with tile.TileContext(nc) as tc:
        with contextlib.ExitStack() as ctx:
            _build(ctx, tc, io)
    nc.compile()
    _CACHE["nc"] = nc
    return nc


def kernel(x, w_q, w_k, w_v, w_o, b_o, **run_kwargs):
    nc = build_nc()
    x = np.asarray(x, dtype=np.float32)
    shared = {
        "wq_t": np.ascontiguousarray(np.asarray(w_q, np.float32).T),
        "wk_t": np.ascontiguousarray(np.asarray(w_k, np.float32).T),
        "wv_t": np.ascontiguousarray(np.asarray(w_v, np.float32).T),
        "wo_t": np.ascontiguousarray(np.asarray(w_o, np.float32).T),
        "b_o": np.asarray(b_o, np.float32),
        "ones_d": np.ones(8, np.float32),
    }
    in_maps = [
        {"x_t": np.ascontiguousarray(x[i].T), **shared} for i in range(8)
    ]
    res = run_bass_kernel_spmd(nc, in_maps, core_ids=list(range(8)),
                               **run_kwargs)
    out = np.stack([res.results[i]["y"] for i in range(8)], axis=0)
    if run_kwargs:
        kernel.last_result = res
    return out
